# revision 30
# baseline (speedup 1.0000x reference)
"""Trainium2 Bass kernel for nn_Blast: out = x @ (W0 + 1 bias^T) + bias
where W0 block (i_in, i_out) = Vt[i] @ diag(S[o,i]) @ U[o].

v12: y-factorization, token-half pipeline, single-queue DMA.

Per core (256 tokens, 2 halves of 128):
  phase A_h: yT[(i,r), t] = blockdiag(Vt)^T @ xT_h  (32 mm, M=32 strips
             via tile_position col groups; col 16 = ones -> rowsum)
  z_h:       zT[(o,r), t] = smat^T @ ysb_h          (16 mm, bank-major)
  phase B_h: out_h = zsb_h vs usb matmuls           (16 mm)
B(h0) copies + out0 DMA overlap x(h1); z bank b feeds B bank b.

Empirical HW laws this is built around:
 - DMA receipts lag bytes-done by ~1-5us, growing with the number of
   in-flight DMAs (sem-lane reuse also stalls issues past ~10). So ALL
   inputs ride ONE ring (sync) in dependency order: wpA(vt+uc), x0 a/b,
   sm, ones, x1 a/b, bias — receipts then fire in order. Out pieces
   follow on the same ring; scalar ring carries no DMA at all.
 - Engine queues are strictly in-order; copy-engine programs interleave
   B(h0) copies with y(h1) copies. usb build (4 rearranged DVE copies)
   runs right after the wpA receipt, before any y copy.
 - matmul start=True clears the whole PSUM bank's has_written: groups
   sharing a bank never interleave.
 - Partition access: start 32-aligned; non-zero start spans <= 32 rows.
   Const-1 row lives at ysb row 127 so the tau0 y-copy splits in two
   ([0:96], [96:127]); a [1,256] ones DMA fills row 127 once.
 - Framework postamble ~8.3us after last DMA receipt (fixed).
bias trick: uc row 16 carries bias for o_loc=0 blocks; a strided DMA
fills row 16 of o_loc=1..3 col-blocks (disjoint from all copies).
Out copies are [128,1024] (2 PSUM banks per po tile, one copy + one DMA
piece per o-bank). z copies are [128,256] bank-pairs.
"""

import numpy as np

IN_DIM = 4096
OUT_DIM = 4096
BLOCK = 256
RANK = 16
B_IN = 16
B_OUT = 16
N_CORES = 8
TOK = 2048
TPC = TOK // N_CORES          # 256 tokens per core
HTOK = 128                    # tokens per half
NCHUNK = IN_DIM // 128        # 32 K-chunks
NTAU = 4                      # y tiles (4 i-blocks each)
NBANK = 4                     # z/usb banks: 4 o-blocks each, pitch 32
BROW = 16                     # bias/rowsum row in zsb/usb (per bank)
CROW = 127                    # const-1 row (ysb, tau0 block)
XB_CH = 16                    # chunks per x batch (2 batches per half)
NWARM = 26                    # K=128 N=256 warm matmuls (HAM un-throttle)
NFILL = 8                     # N=64 fillers at the x batch boundary

VT_C = NCHUNK * 32            # 1024
SM_C = NTAU * NBANK * 128     # 2048 (bank-major blocks: idx NTAU*b+tau)
UC_C = NBANK * BLOCK          # 1024
WPA_C = VT_C + UC_C           # 2048

_CACHE = {}

# test.py toggles; harness never touches these
TRACE = False
TRACE_DIR = None
LAST_RESULTS = None


def _bank(o):
    return o // 4, o % 4


def build_program():
    import concourse.mybir as mybir
    from concourse import bacc
    from concourse.tile import TileContext

    bf16 = mybir.dt.bfloat16
    f32 = mybir.dt.float32

    nc = bacc.Bacc(trn_type="TRN2")
    # xt[p, half*4096 + k*128 + t] = x^T[128k+p, 128*half + t]
    xt_d = nc.dram_tensor("xt", (128, NCHUNK * TPC), bf16, kind="ExternalInput")
    vt_d = nc.dram_tensor("vt", (128, VT_C), bf16, kind="ExternalInput")
    uc_d = nc.dram_tensor("uc", (128, UC_C), bf16, kind="ExternalInput")
    sm_d = nc.dram_tensor("sm", (128, SM_C), bf16, kind="ExternalInput")
    br_d = nc.dram_tensor("br", (1, 12 * BLOCK), bf16, kind="ExternalInput")
    ones_d = nc.dram_tensor("ones", (1, TPC), bf16, kind="ExternalInput")
    out_d = nc.dram_tensor("out", (TPC, OUT_DIM), bf16, kind="ExternalOutput")

    with TileContext(nc) as tc:
        from contextlib import ExitStack

        with ExitStack() as ctx:
            consts = ctx.enter_context(tc.tile_pool(name="consts", bufs=1))
            xpool = ctx.enter_context(tc.tile_pool(name="xpool", bufs=1))
            outsb = ctx.enter_context(tc.tile_pool(name="outsb", bufs=1))
            ps_y = ctx.enter_context(tc.tile_pool(name="ps_y", bufs=1, space="PSUM"))
            ps_z = ctx.enter_context(tc.tile_pool(name="ps_z", bufs=1, space="PSUM"))

            # ---- SBUF tiles ----
            wsrc = consts.tile([128, TPC], bf16, name="wsrc", tag="wsrc")
            nc.vector.memset(wsrc[:], 0.0)

            vt_sb2 = consts.tile([128, VT_C], bf16, name="vt_sb", tag="vt_sb")
            uc_sb2 = consts.tile([128, UC_C], bf16, name="uc_sb", tag="uc_sb")
            vt_sb = vt_sb2[:]
            uc_sb = uc_sb2[:]
            smat_sb = consts.tile([128, SM_C], bf16, name="smat_sb", tag="smat_sb")
            usb = consts.tile([128, OUT_DIM], bf16, name="usb", tag="usb")
            ysb = consts.tile([128, NTAU * TPC], bf16, name="ysb", tag="ysb")
            zsb = consts.tile([128, NBANK * TPC], bf16, name="zsb", tag="zsb")

            # usb zeroing split across DVE and GPS (memsets run at ~1x)
            nc.vector.memset(usb[:, 0:2048], 0.0)
            nc.gpsimd.memset(usb[:, 2048:4096], 0.0)

            usb_v = usb[:].rearrange("p (b oq q) -> p b oq q", b=NBANK, q=BLOCK)

            # ---- input DMAs: ONE ring (sync), dependency order ----
            xbat = {}

            def xdma(h, bi, eng):
                xb = xpool.tile(
                    [128, XB_CH * HTOK], bf16,
                    name=f"xb{h}_{bi}", tag=f"xb{h}_{bi}",
                )
                c0 = h * NCHUNK * HTOK + bi * XB_CH * HTOK
                eng.dma_start(out=xb[:], in_=xt_d[:, c0 : c0 + XB_CH * HTOK])
                xbat[(h, bi)] = xb

            # ONE ring (sync), dependency order: concurrent queues would
            # steal early bandwidth from x0; receipts fire in FIFO order.
            # ones/uc lead (tiny + unblock usb build), vt before x0.
            nc.sync.dma_start(out=ysb[CROW : CROW + 1, 0:TPC], in_=ones_d[:])
            nc.sync.dma_start(out=uc_sb2[:], in_=uc_d[:])
            nc.sync.dma_start(out=vt_sb2[:], in_=vt_d[:])
            xdma(0, 0, nc.sync)
            xdma(0, 1, nc.sync)
            nc.sync.dma_start(out=smat_sb[:], in_=sm_d[:])
            nc.sync.dma_start(
                out=usb_v[BROW : BROW + 1, :, 1:4, :],
                in_=br_d[:].rearrange("p (b oq q) -> p b oq q", b=NBANK, q=BLOCK),
            )
            xdma(1, 0, nc.sync)
            xdma(1, 1, nc.sync)

            def xchunk(h, k):
                xb = xbat[(h, k // XB_CH)]
                return xb[:, (k % XB_CH) * HTOK : (k % XB_CH + 1) * HTOK]

            # ---- usb build: 4 rearranged DVE copies (one per o_loc) ----
            uc_v = uc_sb.rearrange("p (b q) -> p b q", b=NBANK)
            for o_loc in range(4):
                r0 = 32 * o_loc
                nc.vector.tensor_copy(
                    usb_v[r0 : r0 + 32, :, o_loc, :],
                    uc_v[r0 : r0 + 32, :, :],
                )

            # PSUM: 2 y banks + 2 z banks + 4 out banks = 8 exactly.
            ypair = [
                ps_y.tile([128, 2 * TPC], f32, name=f"yp{t}", tag=f"yp{t}")
                for t in range(2)
            ]
            zpair = [
                ps_z.tile([128, 2 * TPC], f32, name=f"zp{t}", tag=f"zp{t}")
                for t in range(2)
            ]

            def ytile(tau, h):
                return ypair[tau // 2][
                    :, TPC * (tau % 2) + HTOK * h : TPC * (tau % 2) + HTOK * (h + 1)
                ]

            def ztile(b, h):
                # pair p=b//2 tile; col 256*h + 128*(b%2)
                return zpair[b // 2][
                    :, 256 * h + 128 * (b % 2) : 256 * h + 128 * (b % 2) + 128
                ]

            def zsb_col(b, h):
                return 512 * (b // 2) + 256 * h + 128 * (b % 2)

            # ---- PE warmup ----
            warm = zpair[1][:, 0:TPC]
            for _ in range(NWARM):
                nc.tensor.matmul(
                    warm, lhsT=wsrc[:, 0:128], rhs=wsrc[:],
                    start=True, stop=True, tile_position=(0, 0),
                )

            def filler(n):
                for _ in range(n):
                    nc.tensor.matmul(
                        warm[0:128, 0:128], lhsT=wsrc[:, 0:128],
                        rhs=wsrc[:, 0:128],
                        start=True, stop=True, tile_position=(0, 0),
                    )

            ps_out = ctx.enter_context(
                tc.tile_pool(name="ps_out", bufs=2, space="PSUM")
            )
            osb = [
                outsb.tile([128, OUT_DIM], bf16, name=f"osb{h}", tag=f"osb{h}")
                for h in range(2)
            ]

            # ---------------- PE pieces ----------------
            def phase_a(h):
                for k in range(NCHUNK):
                    i = k // 2
                    tau, j = i // 4, i % 4
                    nc.tensor.matmul(
                        ytile(tau, h)[32 * j : 32 * j + 32, :],
                        lhsT=vt_sb[:, 32 * k : 32 * k + 32],
                        rhs=xchunk(h, k),
                        start=(k % 2 == 0),
                        stop=(k % 2 == 1),
                        tile_position=(0, 32 * j),
                    )
                    if k == XB_CH - 1:
                        filler(NFILL)

            def z_bank(h, b):
                for tau in range(NTAU):
                    nc.tensor.matmul(
                        ztile(b, h),
                        lhsT=smat_sb[
                            :, 128 * (NTAU * b + tau) : 128 * (NTAU * b + tau + 1)
                        ],
                        rhs=ysb[:, TPC * tau + HTOK * h : TPC * tau + HTOK * (h + 1)],
                        start=(tau == 0),
                        stop=(tau == NTAU - 1),
                        tile_position=(0, 0),
                    )

            def b_bank(h, b):
                po = ps_out.tile([128, 1024], f32, name="po", tag="po")
                for o in range(4 * b, 4 * b + 4):
                    nc.tensor.matmul(
                        po[:, (o % 4) * BLOCK : (o % 4 + 1) * BLOCK],
                        lhsT=zsb[:, zsb_col(b, h) : zsb_col(b, h) + 128],
                        rhs=usb[:, BLOCK * o : BLOCK * (o + 1)],
                        start=True, stop=True, tile_position=(0, 0),
                    )
                return po

            # ---------------- copies (engine-explicit) ----------------
            def y_copy(h, t, eng):
                if t == 0:
                    eng(ysb[0:96, HTOK * h : HTOK * (h + 1)], ytile(0, h)[0:96, :])
                    eng(
                        ysb[96:CROW, HTOK * h : HTOK * (h + 1)],
                        ytile(0, h)[96:CROW, :],
                    )
                else:
                    eng(
                        ysb[:, TPC * t + HTOK * h : TPC * t + HTOK * (h + 1)],
                        ytile(t, h),
                    )

            def z_copy_pair(h, p, eng):
                # banks 2p, 2p+1 for half h are adjacent 256 cols
                eng(
                    zsb[:, 512 * p + 256 * h : 512 * p + 256 * (h + 1)],
                    zpair[p][:, 256 * h : 256 * (h + 1)],
                )

            def o_copy(h, b, po, eng):
                eng(osb[h][:, 1024 * b : 1024 * (b + 1)], po[:])

            def o_piece(h, b):
                c0 = 1024 * b
                nc.sync.dma_start(
                    out=out_d[HTOK * h : HTOK * (h + 1), c0 : c0 + 1024],
                    in_=osb[h][:, c0 : c0 + 1024],
                )

            dve = nc.vector.tensor_copy
            act = nc.scalar.copy

            def ys(h):
                y_copy(h, 0, dve)
                y_copy(h, 1, act)
                y_copy(h, 2, dve)
                y_copy(h, 3, act)

            def zb_front(h):
                z_bank(h, 0)
                z_bank(h, 1)
                z_copy_pair(h, 0, dve)
                po0 = b_bank(h, 0)
                z_bank(h, 2)
                z_bank(h, 3)
                z_copy_pair(h, 1, act)
                o_copy(h, 0, po0, dve)
                o_piece(h, 0)
                po1 = b_bank(h, 1)
                o_copy(h, 1, po1, act)
                o_piece(h, 1)

            def zb_back(h, last=False):
                po2 = b_bank(h, 2)
                o_copy(h, 2, po2, dve)
                o_piece(h, 2)
                po3 = b_bank(h, 3)
                o_copy(h, 3, po3, act)
                if not last:
                    o_piece(h, 3)
                else:
                    # small final piece -> faster last receipt
                    nc.sync.dma_start(
                        out=out_d[HTOK * h : HTOK * (h + 1), 3072:3840],
                        in_=osb[h][:, 3072:3840],
                    )
                    nc.sync.dma_start(
                        out=out_d[HTOK * h : HTOK * (h + 1), 3840:4096],
                        in_=osb[h][:, 3840:4096],
                    )

            # Pipeline, with wait_until stamps (virtual-time floors) to
            # stop the scheduler from hoisting half-1 work ahead of
            # half-0's output chain (its cost model thinks DMAs land
            # instantly). A1 + y1 copies slot between B0's banks so the
            # copy engines pick them up between h0 out-copies.
            phase_a(0)
            filler(8)
            ys(0)
            zb_front(0)
            with tc.tile_wait_until(0.013):
                phase_a(1)
                filler(8)
                ys(1)
            zb_back(0)
            with tc.tile_wait_until(0.016):
                zb_front(1)
                zb_back(1, last=True)

    nc.compile()
    return nc


def prep_inputs(x, S, U, Vt, bias):
    """Host-side layout prep (bf16). Returns per-core input maps."""
    import ml_dtypes

    bf = ml_dtypes.bfloat16
    x = np.asarray(x, dtype=np.float32)
    S = np.asarray(S, dtype=np.float32)
    U = np.asarray(U, dtype=np.float32)
    Vt = np.asarray(Vt, dtype=np.float32)
    bias = np.asarray(bias, dtype=np.float32)

    xt = np.ascontiguousarray(x.reshape(TOK, IN_DIM).T).astype(bf)  # (4096, 2048)

    # vt[p, 32k + c]: c<16 -> Vt[i, 128h+p, c] (k=2i+h); c==16 -> 1.0; else 0
    vt_host = np.zeros((128, NCHUNK, 32), np.float32)
    for k in range(NCHUNK):
        i, h = k // 2, k % 2
        vt_host[:, k, 0:RANK] = Vt[i, 128 * h : 128 * (h + 1), :]
        vt_host[:, k, 16] = 1.0
    vt_host = vt_host.reshape(128, VT_C)

    # uc[32 o_loc + r, 256 b + q] = U[o, r, q]; row 16 of block b carries
    # bias for o = 4b (the o_loc=0 usb copy places it)
    uc = np.zeros((128, UC_C), np.float32)
    for o in range(B_OUT):
        b, o_loc = _bank(o)
        uc[32 * o_loc : 32 * o_loc + RANK, BLOCK * b : BLOCK * (b + 1)] = U[o]
    for b in range(NBANK):
        uc[BROW, BLOCK * b : BLOCK * (b + 1)] = bias[
            BLOCK * 4 * b : BLOCK * (4 * b + 1)
        ]
    vt_host = vt_host.astype(bf)
    uc = uc.astype(bf)

    # smat bank-major: block (b, tau) at cols 128*(NTAU*b + tau);
    # const-1 source row is CROW (=127) of the tau=0 block
    smat = np.zeros((128, SM_C), np.float32)
    for b in range(NBANK):
        for tau in range(NTAU):
            c0 = 128 * (NTAU * b + tau)
            for j in range(4):
                i = 4 * tau + j
                for o in range(4 * b, 4 * b + 4):
                    o_loc = o % 4
                    for r in range(RANK):
                        smat[32 * j + r, c0 + 32 * o_loc + r] = S[o, i, r]
                smat[32 * j + 16, c0 + BROW] = 1.0
            if tau == 0:
                smat[CROW, c0 + BROW] = 1.0
    smat = smat.astype(bf)

    # br2[b, o_loc-1, q] = bias[256*(4b + o_loc) + q] for o_loc 1..3
    br2 = np.zeros((NBANK, 3, BLOCK), np.float32)
    for b in range(NBANK):
        for ol in (1, 2, 3):
            br2[b, ol - 1] = bias[BLOCK * (4 * b + ol) : BLOCK * (4 * b + ol + 1)]
    br2 = br2.reshape(1, 12 * BLOCK).astype(bf)

    ones = np.ones((1, TPC), np.float32).astype(bf)

    in_maps = []
    for c in range(N_CORES):
        xc = (
            xt[:, c * TPC : (c + 1) * TPC]
            .reshape(NCHUNK, 128, 2, HTOK)
            .transpose(1, 2, 0, 3)
            .reshape(128, NCHUNK * TPC)
        )
        in_maps.append(
            {
                "xt": np.ascontiguousarray(xc),
                "vt": vt_host,
                "uc": uc,
                "sm": smat,
                "br": br2,
                "ones": ones,
            }
        )
    return in_maps


def kernel(x, S, U, Vt, bias):
    global LAST_RESULTS
    from concourse.bass_utils import run_bass_kernel_spmd

    if "nc" not in _CACHE:
        _CACHE["nc"] = build_program()
    nc = _CACHE["nc"]

    in_maps = prep_inputs(x, S, U, Vt, bias)
    res = run_bass_kernel_spmd(
        nc, in_maps, list(range(N_CORES)), trace=TRACE, tmpdir=TRACE_DIR
    )
    LAST_RESULTS = res
    out = np.concatenate(
        [np.asarray(res.results[c]["out"]).astype(np.float32) for c in range(N_CORES)],
        axis=0,
    )
    return out.reshape(2, TOK // 2, OUT_DIM)
